# revision 36
# baseline (speedup 1.0000x reference)
"""BertBiAttention Trainium2 kernel (v2).

Cross-attention between two streams (B=4, S=2048, HID=768, H=12 heads).
Sharding: 8 cores = (stream s in {1,2}) x (batch b in {0..3}). Each core
computes one stream's full output for one batch element:
    h_s[b] = LayerNorm( attend(q_other, k_own, v_own, mask_own) @ wd + bd + x_own )
No collectives needed; the host stacks per-core outputs.

v2 structure (per core; ~604us vs 832us baseline, ACT(softmax-exp)-bound):
  - scores: bf16 row-tiled CONCURRENT matmul pairs -- heads (2f, 2f+1) live at
    partition rows 0:64 / 64:128 of qT/kT[f]; tile_position (0,0)/(64,0) runs
    both K=64 matmuls simultaneously on the PE array (~2x score throughput).
  - softmax exp on ACT writes fp8e4 directly; ctx matmuls are fp8 DoubleRow
    (two key-tiles per pass, ~1.8x) with lhsT = packed v [128, 2, 12, 80]
    ([v*exp(mask) | exp(mask)] per head, denom row included, M=65).
  - q/k/v projections are fp8 DoubleRow over fp8-transposed x; weights are
    scaled by 16 into e4m3's normal range and the 1/16 is folded into the
    DVE bias-add / the emask multiplier.
  - ctx evicted into ctx2 [128, 6, 512] (heads packed along partitions via
    DVE partition-base-shifted copies) so dense runs K=128 matmuls (2x);
    denominators batched: reciprocal_approx_fast + bf16 DRAM-bounce
    broadcast + one in-place 2x-mode multiply per f-tile.
  - aggressive software pipelining around the in-order engine queues: each
    pair's last two ctx groups + PSUM evict are emitted during the next
    pair's first steps; kv/q transposes, kT/qT projections, v projection,
    dense+LN all run as fill work inside the ACT-bound attention phase
    (deterministic pre-step lists where ordering is correctness-critical).
  - startup trimmed to ~14us: x rows prefetched before the ~0.6us-each
    constant DMAs (batched into single strided transfers), weight casts on
    the idle ACT engine, only chunk-0 transposes + kT[0]/qT[0] before the
    first score matmul.
  - dense + residual + LayerNorm (rstd = exp(-0.5*ln(var+eps)) keeps ACT on
    one table set) deferred as fill work into the next chunk's attention.
"""

import numpy as np

import concourse.bass as bass
import concourse.mybir as mybir
import concourse.tile as tile
from concourse import bacc, bass_utils
from concourse.masks import make_identity

B, S, HID, H, HD = 4, 2048, 768, 12, 64
FT = HID // 128   # 6 feature tiles
ST = S // 128     # 16 seq tiles
QT = S // 512     # 4 q chunks
NH = 2            # 768-wide outputs split into 2 x 384
NW = 384
VW = 80           # padded per-head width in packed v (64 ctx + 1 denom + pad)
EPS = 1e-12

F32 = mybir.dt.float32
BF16 = mybir.dt.bfloat16
FP8 = mybir.dt.float8e4
I16 = mybir.dt.int16

# Schraudolph fast-exp in bf16 bit domain: bits = s*(0.125*log2e*128) +
# (127*128 - c); bitcast int16 -> bf16 gives ~exp(s/8) (std ~1.8%, mean
# bias zeroed via c; the constant part cancels in softmax normalization)
SCH_A = 0.125 * 1.4426950408889634 * 128.0
SCH_B = 127.0 * 128.0 - 7.34
DVE_U = ()  # DVE softmax-exp offload disabled: ACT win < DVE FIFO coupling cost
AF = mybir.ActivationFunctionType
DR = mybir.MatmulPerfMode.DoubleRow


def _bcast_part(ap, p=128):
    """DRAM row [1, N] -> partition-broadcast AP [p, N] (stride-0 partition)."""
    return bass.AP(tensor=ap.tensor, offset=ap.offset, ap=[[0, p], ap.ap[-1]])


def _bcast_pair(ap2):
    """DRAM [2, N] -> [2, 64, N] AP (stride-0 middle): one DMA broadcasts
    both heads' reciprocal rows onto partitions 0:64 / 64:128."""
    return bass.AP(
        tensor=ap2.tensor, offset=ap2.offset,
        ap=[ap2.ap[0], [0, 64], ap2.ap[-1]],
    )


def build_nc():
    nc = bacc.Bacc("TRN2", target_bir_lowering=False, debug=False, num_devices=8)

    xq_d = nc.dram_tensor("xq", [S, HID], F32, kind="ExternalInput").ap()
    xkv_d = nc.dram_tensor("xkv", [S, HID], F32, kind="ExternalInput").ap()
    wq_d = nc.dram_tensor("wq", [HID, HID], F32, kind="ExternalInput").ap()
    wk_d = nc.dram_tensor("wk", [HID, HID], F32, kind="ExternalInput").ap()
    wv_d = nc.dram_tensor("wv", [HID, HID], F32, kind="ExternalInput").ap()
    wd_d = nc.dram_tensor("wd", [HID, HID], F32, kind="ExternalInput").ap()
    bq_d = nc.dram_tensor("bq", [1, HID], F32, kind="ExternalInput").ap()
    bk_d = nc.dram_tensor("bk", [1, HID], F32, kind="ExternalInput").ap()
    bv_d = nc.dram_tensor("bv", [1, HID], F32, kind="ExternalInput").ap()
    bd_d = nc.dram_tensor("bd", [1, HID], F32, kind="ExternalInput").ap()
    mask_d = nc.dram_tensor("mask", [S, 1], F32, kind="ExternalInput").ap()
    lng_d = nc.dram_tensor("lng", [1, HID], F32, kind="ExternalInput").ap()
    lnb_d = nc.dram_tensor("lnb", [1, HID], F32, kind="ExternalInput").ap()
    out_d = nc.dram_tensor("out", [S, HID], F32, kind="ExternalOutput").ap()

    with tile.TileContext(nc) as tc:
        with (
            tc.tile_pool(name="consts", bufs=1) as consts,
            tc.tile_pool(name="big", bufs=1) as big,
        ):
            # ---- constants (batched single DMAs) ----
            ident = consts.tile([128, 128], F32)
            make_identity(nc, ident)
            ones_r = consts.tile([1, 128], BF16)
            nc.vector.memset(ones_r, 1.0)
            ones_12 = consts.tile([128, 12], F32)
            nc.vector.memset(ones_12, 1.0)
            eps_t = consts.tile([128, 1], F32)
            nc.vector.memset(eps_t, EPS)

            bqc = consts.tile([128, FT], F32)
            bkc = consts.tile([128, FT], F32)
            nc.sync.dma_start(
                out=bqc,
                in_=bq_d[0:1, :].rearrange("a (f p) -> (a p) f", p=128),
            )
            nc.sync.dma_start(
                out=bkc,
                in_=bk_d[0:1, :].rearrange("a (f p) -> (a p) f", p=128),
            )
            bv_f = consts.tile([1, HID], F32)
            nc.sync.dma_start(out=bv_f, in_=bv_d)
            bd_f = consts.tile([1, HID], F32)
            nc.sync.dma_start(out=bd_f, in_=bd_d)
            # v projection runs in fp8 with weights scaled by 16; bias is
            # added inside the PSUM accumulation, so pre-scale it too
            bv_row = consts.tile([1, HID], BF16)
            nc.vector.tensor_scalar_mul(out=bv_row, in0=bv_f, scalar1=16.0)
            bd_row = consts.tile([1, HID], BF16)
            nc.vector.tensor_copy(out=bd_row, in_=bd_f)

            mask_t = consts.tile([128, ST], F32)
            nc.sync.dma_start(
                out=mask_t,
                in_=mask_d[:, 0:1].rearrange("(t p) a -> p (t a)", p=128),
            )
            emask = consts.tile([128, ST], F32)
            nc.scalar.activation(out=emask, in_=mask_t, func=AF.Exp)
            emask16 = consts.tile([128, ST], F32)
            nc.vector.tensor_scalar_mul(out=emask16, in0=emask, scalar1=1.0 / 16.0)

            # broadcast ln gamma/beta to all 128 partitions (stride-0 DMA)
            g_bc = consts.tile([128, HID], F32)
            b_bc = consts.tile([128, HID], F32)
            nc.sync.dma_start(out=g_bc, in_=_bcast_part(lng_d))
            nc.sync.dma_start(out=b_bc, in_=_bcast_part(lnb_d))

            # ---- persistent activation buffers ----
            qT = [big.tile([128, S], BF16, name=f"qT{f}") for f in range(FT)]
            kT = [big.tile([128, S], BF16, name=f"kT{f}") for f in range(FT)]
            # packed v, fp8: [keys, ktile-pair-half, head, 64 ctx + denom]
            vb_dr = [
                big.tile([128, 2, H, VW], FP8, name=f"vbdr{u}") for u in range(8)
            ]
            # wd natural rows, bf16 (dense is K=128 over packed ctx2)
            dw6 = [big.tile([128, HID], BF16, name=f"dw6{f}") for f in range(FT)]
            # transposed x_kv chunks stay alive for the deferred v projection
            xTkv = [
                big.tile([128, FT, 512], FP8, name=f"xTkv{c}") for c in range(QT)
            ]
            # q/k/v projection weights: fp8, scaled by 16 (keeps the ~N(0,
            # 0.02) values out of e4m3's subnormal range), packed as
            # [hid_in, 2 k-subtiles, hid_out] for DoubleRow matmuls
            wv_p = [big.tile([128, 2, HID], FP8, name=f"wvp{j}") for j in range(3)]
            wq_p = [big.tile([128, 2, HID], FP8, name=f"wqp{j}") for j in range(3)]
            wk_p = [big.tile([128, 2, HID], FP8, name=f"wkp{j}") for j in range(3)]

            # ---- startup: transpose x_kv, project kT (v deferred) ----
            with (
                tc.tile_pool(name="xn2", bufs=3) as xn2_pool,
                tc.tile_pool(name="ps_tp2", bufs=2, space="PSUM") as ps_tp2,
                tc.tile_pool(name="ps_pj2", bufs=2, space="PSUM") as ps_pj2,
            ):
                # prefetch chunk-0 x rows first: every DMA costs ~0.6us of
                # Sync queue time; this is the startup critical path
                xpre = xn2_pool.tile([128, 8, HID], F32, name="xpre")
                for ss in range(4):
                    nc.sync.dma_start(
                        out=xpre[:, ss, :],
                        in_=xkv_d[ss * 128 : (ss + 1) * 128, :],
                    )
                for ss in range(4):
                    nc.sync.dma_start(
                        out=xpre[:, 4 + ss, :],
                        in_=xq_d[ss * 128 : (ss + 1) * 128, :],
                    )

                def wpack(dst, src_d):
                    # casts run on ACT (idle during startup; the DVE queue is
                    # the startup critical path with the transpose evictions)
                    for j in range(3):
                        for i in range(2):
                            f = 2 * j + i
                            wtmp = xn2_pool.tile([128, HID], F32, name="wtmp2")
                            nc.sync.dma_start(
                                out=wtmp, in_=src_d[f * 128 : (f + 1) * 128, :]
                            )
                            nc.scalar.mul(dst[j][:, i, :], wtmp, 16.0)

                wpack(wk_p, wk_d)
                wpack(wq_p, wq_d)

                # only chunk 0 of x_kv is transposed before attention starts;
                # chunks 1-3 stream in as pair-0 pre-steps (scores consume
                # kT[0] chunk c only from kc=4c, so the work can lag)
                for chunk in range(1):
                    xT_c = xTkv[chunk]
                    for ss in range(4):
                        x_nat = xpre[:, ss, :]
                        # 6 transposes into one PSUM tile, ONE batched DVE
                        # eviction (the per-tile copies were the startup
                        # critical path: 6x290ns -> 1x960ns)
                        tp6 = ps_tp2.tile([128, FT, 128], F32, name="tp_ps")
                        for f in range(FT):
                            nc.tensor.transpose(
                                tp6[:, f, :], x_nat[:, f * 128 : (f + 1) * 128],
                                ident,
                            )
                        nc.vector.tensor_copy(
                            out=xT_c[:, :, ss * 128 : (ss + 1) * 128], in_=tp6
                        )
                    # only kT[0] inline (pair 0 needs it immediately);
                    # fo 1..5 are deferred as attention-phase fill work
                    pj = ps_pj2.tile([128, 512], F32, name="pj2")
                    for j in range(3):
                        nc.tensor.matmul(
                            pj,
                            wk_p[j][:, :, 0:128],
                            xT_c[:, 2 * j : 2 * j + 2, :],
                            start=(j == 0),
                            stop=(j == 2),
                            perf_mode=DR,
                        )
                    nc.vector.tensor_scalar(
                        out=kT[0][:, chunk * 512 : (chunk + 1) * 512],
                        in0=pj,
                        scalar1=1.0 / 16.0,
                        scalar2=bkc[:, 0:1],
                        op0=mybir.AluOpType.mult,
                        op1=mybir.AluOpType.add,
                    )

                # q-side chunk 0: batched transposes + fo=0 projection
                xT0 = big.tile([128, FT, 512], FP8, name="xT0")
                for ss in range(4):
                    tp6 = ps_tp2.tile([128, FT, 128], F32, name="tp_ps")
                    for f in range(FT):
                        nc.tensor.transpose(
                            tp6[:, f, :],
                            xpre[:, 4 + ss, f * 128 : (f + 1) * 128],
                            ident,
                        )
                    nc.vector.tensor_copy(
                        out=xT0[:, :, ss * 128 : (ss + 1) * 128], in_=tp6
                    )
                pj = ps_pj2.tile([128, 512], F32, name="pj2")
                for j in range(3):
                    nc.tensor.matmul(
                        pj,
                        wq_p[j][:, :, 0:128],
                        xT0[:, 2 * j : 2 * j + 2, :],
                        start=(j == 0),
                        stop=(j == 2),
                        perf_mode=DR,
                    )
                nc.vector.tensor_scalar(
                    out=qT[0][:, 0:512],
                    in0=pj,
                    scalar1=1.0 / 16.0,
                    scalar2=bqc[:, 0:1],
                    op0=mybir.AluOpType.mult,
                    op1=mybir.AluOpType.add,
                )

                # wv casts last: v projection first consumes them ~17us in
                wpack(wv_p, wv_d)

            # ---- attention + dense + layernorm ----
            with (
                tc.tile_pool(name="xnq", bufs=3) as xnq_pool,
                tc.tile_pool(name="xTq", bufs=2) as xTq_pool,
                tc.tile_pool(name="ctx2_pool", bufs=2) as ctx2_pool,
                tc.tile_pool(name="exp_pool", bufs=5) as exp_pool,
                tc.tile_pool(name="exp16_pool", bufs=2) as exp16_pool,
                tc.tile_pool(name="dram_pool", bufs=2, space="DRAM") as dram_pool,
                tc.tile_pool(name="rec_pool", bufs=2) as rec_pool,
                tc.tile_pool(name="res_pool", bufs=2) as res_pool,
                tc.tile_pool(name="dt_pool", bufs=1) as dt_pool,
                tc.tile_pool(name="hpre_pool", bufs=1) as hpre_pool,
                tc.tile_pool(name="st_pool", bufs=2) as st_pool,
                tc.tile_pool(name="ps_sc", bufs=2, space="PSUM") as ps_sc,
                tc.tile_pool(name="ps_ctx", bufs=1, space="PSUM") as ps_ctx,
                tc.tile_pool(name="ps_aux", bufs=2, space="PSUM") as ps_aux,
            ):
                def wload_fill(dst, src_d, f3):
                    def run():
                        for f in f3:
                            wtmp = xnq_pool.tile([128, HID], F32, name="x_nat")
                            nc.sync.dma_start(
                                out=wtmp, in_=src_d[f * 128 : (f + 1) * 128, :]
                            )
                            nc.vector.tensor_copy(out=dst[f], in_=wtmp)

                    return run

                def trkv_fill(chunk, ss):
                    def run():
                        x_nat = xnq_pool.tile([128, HID], F32, name="x_nat")
                        st = chunk * 4 + ss
                        nc.sync.dma_start(
                            out=x_nat, in_=xkv_d[st * 128 : (st + 1) * 128, :]
                        )
                        q_transpose_ss(xTkv[chunk], x_nat, ss)

                    return run

                def kt_fill(fo, chunk):
                    def run():
                        pj = ps_aux.tile([128, 512], F32, name="aux")
                        for j in range(3):
                            nc.tensor.matmul(
                                pj,
                                wk_p[j][:, :, fo * 128 : (fo + 1) * 128],
                                xTkv[chunk][:, 2 * j : 2 * j + 2, :],
                                start=(j == 0),
                                stop=(j == 2),
                                perf_mode=DR,
                            )
                        nc.vector.tensor_scalar(
                            out=kT[fo][:, chunk * 512 : (chunk + 1) * 512],
                            in0=pj,
                            scalar1=1.0 / 16.0,
                            scalar2=bkc[:, fo : fo + 1],
                            op0=mybir.AluOpType.mult,
                            op1=mybir.AluOpType.add,
                        )

                    return run

                # --- deferred v projection: one 128-row step, both halves ---
                def v_proj_step(st):
                    u, half = st // 2, st % 2
                    chunk, ss = st // 4, st % 4
                    xT_c = xTkv[chunk]
                    for nh in range(NH):
                        vp = ps_aux.tile([128, 512], F32, name="aux")
                        for j in range(3):
                            nc.tensor.matmul(
                                vp[:, 0:NW],
                                xT_c[:, 2 * j : 2 * j + 2, ss * 128 : (ss + 1) * 128],
                                wv_p[j][:, :, nh * NW : (nh + 1) * NW],
                                start=(j == 0),
                                stop=False,
                                perf_mode=DR,
                            )
                        nc.tensor.matmul(
                            vp[:, 0:NW],
                            ones_r,
                            bv_row[0:1, nh * NW : (nh + 1) * NW],
                            start=False,
                            stop=True,
                        )
                        nc.vector.tensor_scalar_mul(
                            out=vb_dr[u][:, half, nh * 6 : (nh + 1) * 6, 0:HD],
                            in0=vp[:, 0:NW].rearrange("p (a d) -> p a d", a=6),
                            scalar1=emask16[:, st : st + 1],
                        )
                    nc.vector.tensor_scalar_mul(
                        out=vb_dr[u][:, half, :, HD : HD + 1].rearrange(
                            "p a c -> p (a c)"
                        ),
                        in0=ones_12,
                        scalar1=emask[:, st : st + 1],
                    )

                # --- q projection (fill work) ---
                def q_proj_mm(chunk, xT_c, fo_range):
                    for fo in fo_range:
                        pj = ps_aux.tile([128, 512], F32, name="aux")
                        for j in range(3):
                            nc.tensor.matmul(
                                pj,
                                wq_p[j][:, :, fo * 128 : (fo + 1) * 128],
                                xT_c[:, 2 * j : 2 * j + 2, :],
                                start=(j == 0),
                                stop=(j == 2),
                                perf_mode=DR,
                            )
                        nc.vector.tensor_scalar(
                            out=qT[fo][:, chunk * 512 : (chunk + 1) * 512],
                            in0=pj,
                            scalar1=1.0 / 16.0,
                            scalar2=bqc[:, fo : fo + 1],
                            op0=mybir.AluOpType.mult,
                            op1=mybir.AluOpType.add,
                        )

                def q_transpose_ss(xT_c, x_nat, ss):
                    for f0, nf in ((0, 4), (4, 2)):
                        tp_ps = ps_aux.tile([128, 512], F32, name="aux")
                        tpv = tp_ps.rearrange("p (a b) -> p a b", b=128)
                        for j in range(nf):
                            nc.tensor.transpose(
                                tpv[:, j, :],
                                x_nat[:, (f0 + j) * 128 : (f0 + j + 1) * 128],
                                ident,
                            )
                        nc.vector.tensor_copy(
                            out=xT_c[:, f0 : f0 + nf, ss * 128 : (ss + 1) * 128],
                            in_=tpv[:, 0:nf, :],
                        )

                def q_proj_steps(chunk):
                    # DMA prefetch is a separate step popped one slot earlier
                    # than the transposes it feeds, so the in-order PE queue
                    # never stalls on DMA latency
                    state = {}

                    def dstep(ss_pair):
                        def run():
                            if "xT" not in state:
                                state["xT"] = xTq_pool.tile(
                                    [128, FT, 512], FP8, name="xT_q"
                                )
                            for ss in ss_pair:
                                x_nat = xnq_pool.tile([128, HID], F32, name="x_nat")
                                st = chunk * 4 + ss
                                nc.sync.dma_start(
                                    out=x_nat,
                                    in_=xq_d[st * 128 : (st + 1) * 128, :],
                                )
                                state[ss] = x_nat

                        return run

                    def tstep(ss_pair):
                        def run():
                            for ss in ss_pair:
                                q_transpose_ss(state["xT"], state[ss], ss)

                        return run

                    def mstep(fo_range):
                        return lambda: q_proj_mm(chunk, state["xT"], fo_range)

                    return [
                        dstep((0, 1)),
                        tstep((0, 1)),
                        dstep((2, 3)),
                        tstep((2, 3)),
                        mstep(range(0, 2)),
                        mstep(range(2, 4)),
                        mstep(range(4, 6)),
                    ]

                # --- dense + residual + LN for chunk qt (fill work) ---
                def make_dense_steps(qt, ctx2t):
                    state = {}

                    def group_step(ss, nh):
                        def run():
                            if "mvq" not in state:
                                state["mvq"] = st_pool.tile(
                                    [128, 4, 2], F32, name="mvq"
                                )
                                state["hp"] = {}
                            st = qt * 4 + ss
                            ssl = slice(ss * 128, (ss + 1) * 128)
                            if ss not in state["hp"]:
                                state["hp"][ss] = hpre_pool.tile(
                                    [128, HID], F32, name=f"hp{ss}"
                                )
                            hp = state["hp"][ss]
                            h_ps = ps_aux.tile([128, 512], F32, name="aux")
                            for f in range(FT):
                                nc.tensor.matmul(
                                    h_ps[:, 0:NW],
                                    ctx2t[:, f, ssl],
                                    dw6[f][:, nh * NW : (nh + 1) * NW],
                                    start=(f == 0),
                                    stop=False,
                                )
                            nc.tensor.matmul(
                                h_ps[:, 0:NW],
                                ones_r,
                                bd_row[0:1, nh * NW : (nh + 1) * NW],
                                start=False,
                                stop=True,
                            )
                            x_res = res_pool.tile([128, NW], F32, name="x_res")
                            nc.sync.dma_start(
                                out=x_res,
                                in_=xkv_d[
                                    st * 128 : (st + 1) * 128,
                                    nh * NW : (nh + 1) * NW,
                                ],
                            )
                            nc.vector.tensor_add(
                                out=hp[:, nh * NW : (nh + 1) * NW],
                                in0=h_ps[:, 0:NW],
                                in1=x_res,
                            )
                            if nh == NH - 1:
                                stats = st_pool.tile([128, 3, 6], F32, name="stats")
                                for sg in range(3):
                                    nc.vector.bn_stats(
                                        out=stats[:, sg, :],
                                        in_=hp[:, sg * 256 : (sg + 1) * 256],
                                    )
                                nc.vector.bn_aggr(
                                    out=state["mvq"][:, ss, :], in_=stats
                                )

                        return run

                    # (GpSimd LN apply measured ~12us per tile -- too slow;
                    # DVE with the per-ss split keeps FIFO stalls bounded)
                    eng = nc.vector

                    def tail0():
                        mvq = state["mvq"]
                        lnv = st_pool.tile([128, 4], F32, name="lnv")
                        nc.scalar.activation(
                            out=lnv, in_=mvq[:, :, 1], func=AF.Ln,
                            bias=eps_t, scale=1.0,
                        )
                        rstd4 = st_pool.tile([128, 4], F32, name="rstd4")
                        nc.scalar.activation(
                            out=rstd4, in_=lnv, func=AF.Exp, scale=-0.5
                        )
                        state["rstd4"] = rstd4

                    def tail_ss(ss):
                        def run():
                            st = qt * 4 + ss
                            hp = state["hp"][ss]
                            eng.tensor_scalar(
                                out=hp,
                                in0=hp,
                                scalar1=state["mvq"][:, ss, 0:1],
                                scalar2=state["rstd4"][:, ss : ss + 1],
                                op0=mybir.AluOpType.subtract,
                                op1=mybir.AluOpType.mult,
                            )
                            eng.tensor_mul(hp, hp, g_bc)
                            eng.tensor_add(hp, hp, b_bc)
                            nc.sync.dma_start(
                                out=out_d[st * 128 : (st + 1) * 128, :], in_=hp
                            )

                        return run

                    return [
                        group_step(ss, nh) for ss in range(4) for nh in range(NH)
                    ] + [tail0] + [tail_ss(ss) for ss in range(4)]

                pending = []
                pending.append(wload_fill(dw6, wd_d, range(0, 3)))
                pending.append(wload_fill(dw6, wd_d, range(3, 6)))

                def pop_fill():
                    if pending:
                        pending.pop(0)()

                def emit_pair(qt, hp, ctx2t, den_all, v_inline, tails, posts,
                              pre):
                    """Attention for head pair (2hp, 2hp+1), q chunk qt.

                    ctx matmuls lag the exps by ~5 steps; the last two ctx
                    groups + the PSUM evict are RETURNED as closures and
                    emitted during the next pair's first steps, so the
                    in-order PE queue never stalls on the final exps at a
                    pair boundary."""
                    qsl = slice(qt * 512, (qt + 1) * 512)
                    ctxA = ps_ctx.tile([HD + 1, 512], F32, name="ctxA")
                    ctxB = ps_ctx.tile([HD + 1, 512], F32, name="ctxB")
                    exps_u = {}

                    def emit_ctx(u):
                        if u in DVE_U:
                            # bf16 Schraudolph exps: plain-mode matmuls
                            # (fp8 lhsT x bf16 rhs is legal and exact)
                            eb = exps_u[u].bitcast(BF16)
                            for i in range(2):
                                nc.tensor.matmul(
                                    ctxA,
                                    vb_dr[u][:, i, 2 * hp, 0 : HD + 1],
                                    eb[:, 2 * i, :],
                                    start=False, stop=False,
                                )
                                nc.tensor.matmul(
                                    ctxB,
                                    vb_dr[u][:, i, 2 * hp + 1, 0 : HD + 1],
                                    eb[:, 2 * i + 1, :],
                                    start=False, stop=False,
                                )
                            return
                        rv = exps_u[u].rearrange(
                            "p (k two) n -> p two k n", two=2
                        )
                        nc.tensor.matmul(
                            ctxA,
                            vb_dr[u][:, :, 2 * hp, 0 : HD + 1],
                            rv[:, 0],
                            start=(u == 0),
                            stop=(u == 7),
                            perf_mode=DR,
                        )
                        nc.tensor.matmul(
                            ctxB,
                            vb_dr[u][:, :, 2 * hp + 1, 0 : HD + 1],
                            rv[:, 1],
                            start=(u == 0),
                            stop=(u == 7),
                            perf_mode=DR,
                        )

                    for kc in range(ST):
                        u, half = kc // 2, kc % 2
                        sc = ps_sc.tile([128, 2, 512], F32, name="sc")
                        nc.tensor.matmul(
                            sc[:, 0, :],
                            kT[hp][0:HD, kc * 128 : (kc + 1) * 128],
                            qT[hp][0:HD, qsl],
                            start=True, stop=True, tile_position=(0, 0),
                        )
                        nc.tensor.matmul(
                            sc[:, 1, :],
                            kT[hp][HD:128, kc * 128 : (kc + 1) * 128],
                            qT[hp][HD:128, qsl],
                            start=True, stop=True, tile_position=(64, 0),
                        )
                        if v_inline:
                            v_proj_step(kc)
                        if u in DVE_U:
                            if half == 0:
                                exps_u[u] = exp16_pool.tile(
                                    [128, 4, 512], I16, name="exps16"
                                )
                            nc.vector.tensor_scalar(
                                out=exps_u[u][:, 2 * half : 2 * half + 2, :],
                                in0=sc,
                                scalar1=SCH_A,
                                scalar2=SCH_B,
                                op0=mybir.AluOpType.mult,
                                op1=mybir.AluOpType.add,
                            )
                        else:
                            if half == 0:
                                exps_u[u] = exp_pool.tile(
                                    [128, 4, 512], FP8, name="exps"
                                )
                            nc.scalar.activation(
                                out=exps_u[u][:, 2 * half : 2 * half + 2, :],
                                in_=sc, func=AF.Exp, scale=0.125,
                            )
                        if kc in (1, 2) and tails:
                            tails.pop(0)()
                        elif kc == 3 and posts:
                            posts.pop(0)()
                        elif pre:
                            pre.pop(0)()
                            if pre:
                                pre.pop(0)()
                        if kc >= 5 and half == 1:
                            emit_ctx((kc - 5) // 2)
                            if pre:
                                pre.pop(0)()
                            elif kc not in (9, 11):
                                # keep the DVE FIFO clear around the
                                # Schraudolph exps (kc 10-11) so the score
                                # PSUM release isn't delayed behind fills
                                pop_fill()

                    def evict():
                        # ctx rows into packed ctx2 (partition-shift of 64 is
                        # legal on DVE); denom rows via same-partition copy +
                        # DMA (DVE shifts must be multiples of 32)
                        dtmp = dt_pool.tile([HD + 1, 1024], F32, name="dtmp")
                        nc.vector.tensor_copy(
                            out=ctx2t[0:HD, hp, :], in_=ctxA[0:HD, :]
                        )
                        nc.vector.tensor_copy(
                            out=dtmp[HD : HD + 1, 0:512], in_=ctxA[HD : HD + 1, :]
                        )
                        nc.vector.tensor_copy(
                            out=ctx2t[HD:128, hp, :], in_=ctxB[0:HD, :]
                        )
                        nc.vector.tensor_copy(
                            out=dtmp[HD : HD + 1, 512:1024],
                            in_=ctxB[HD : HD + 1, :],
                        )
                        nc.sync.dma_start(
                            out=den_all[2 * hp : 2 * hp + 2, :],
                            in_=dtmp[HD : HD + 1, :],
                        )

                    def norm_pair():
                        # last chunk: normalize this pair during attention so
                        # the post-attention tail only waits on pair 5's norm
                        # (dense f-tiles 0-4 then overlap that chain)
                        # custom-DVE op wants base partition 0: run on the
                        # full tile (untouched rows hold stale but finite
                        # denominators from the prior chunk's buffer)
                        sl = slice(2 * hp, 2 * hp + 2)
                        rec2 = rec_pool.tile([H, 512], F32, name="rec2")
                        nc.vector.reciprocal_approx_fast(
                            out=rec2, in_=den_all
                        )
                        rec2b = rec_pool.tile([H, 512], BF16, name="rec2b")
                        nc.vector.tensor_copy(out=rec2b, in_=rec2)
                        rd = dram_pool.tile([2, 512], BF16, name="rd2")
                        nc.sync.dma_start(out=rd, in_=rec2b[sl, :])
                        bc = rec_pool.tile([128, 512], BF16, name="bc_sb")
                        nc.sync.dma_start(out=bc, in_=_bcast_pair(rd[0:2, :]))
                        nc.vector.tensor_mul(
                            out=ctx2t[:, hp, :],
                            in0=ctx2t[:, hp, :],
                            in1=bc,
                        )

                    def t1():
                        emit_ctx(6)

                    def t2():
                        emit_ctx(7)
                        evict()

                    return [t1, t2]

                def emit_norm(ctx2t, den_all):
                    # batched reciprocal; bf16 partition-broadcast via DRAM
                    # bounce + stride-0 DMA; one in-place 2x multiply per head
                    rec_all = rec_pool.tile([H, 512], F32, name="rec_all")
                    nc.vector.reciprocal_approx_fast(out=rec_all, in_=den_all)
                    rec_bf = rec_pool.tile([H, 512], BF16, name="rec_bf")
                    nc.vector.tensor_copy(out=rec_bf, in_=rec_all)
                    rec_d = dram_pool.tile([H, 512], BF16, name="rec_d")
                    nc.sync.dma_start(out=rec_d, in_=rec_bf)
                    for f in range(FT):
                        # rec rows for heads (2f, 2f+1) stacked on partitions
                        # 0:64 / 64:128 -> one DMA + one multiply per f-tile
                        bc_sb = rec_pool.tile([128, 512], BF16, name="bc_sb")
                        nc.sync.dma_start(
                            out=bc_sb,
                            in_=_bcast_pair(rec_d[2 * f : 2 * f + 2, :]),
                        )
                        nc.vector.tensor_mul(
                            out=ctx2t[:, f, :],
                            in0=ctx2t[:, f, :],
                            in1=bc_sb,
                        )

                tails = []
                posts = []
                for qt in range(QT):
                    if qt + 1 < QT:
                        pending.extend(q_proj_steps(qt + 1))
                    ctx2t = ctx2_pool.tile([128, FT, 512], BF16, name="ctx2")
                    den_all = rec_pool.tile([H, 512], F32, name="den_all")
                    for hp in range(FT):
                        # chunk-0 deferred projections: pair hp emits the
                        # prerequisites of pair hp+1 (its qT f-tile + kT) in
                        # its early steps -- deterministic, never races
                        pre = []
                        if qt == 0 and hp == 0:
                            # stream in chunks 1-3 of x_kv (transposes +
                            # kT[0]) during pair 0; scores only touch kT[0]
                            # chunk c from kc=4c, v only from kc=4c
                            for c in range(1, QT):
                                pre.extend(trkv_fill(c, ss) for ss in range(4))
                                pre.append(kt_fill(0, c))
                        if qt == 0 and hp + 1 < FT:
                            pre.append(
                                lambda fo=hp + 1: q_proj_mm(
                                    0, xT0, range(fo, fo + 1)
                                )
                            )
                            pre.extend(kt_fill(hp + 1, c) for c in range(QT))
                        tails = emit_pair(
                            qt, hp, ctx2t, den_all, qt == 0 and hp == 0,
                            tails, posts, pre,
                        )

                    def post(qt=qt, c=ctx2t, d=den_all):
                        emit_norm(c, d)
                        pending.extend(make_dense_steps(qt, c))

                    posts.append(post)
                for t in tails:
                    t()
                for p in posts:
                    p()
                for step in pending:
                    step()

    nc.compile()
    return nc


_NC = None


def _get_nc():
    global _NC
    if _NC is None:
        _NC = build_nc()
    return _NC


def _prepare(
    input_tensor1, attention_mask1, input_tensor2, attention_mask2,
    q1_w, q1_b, k1_w, k1_b, v1_w, v1_b,
    q2_w, q2_b, k2_w, k2_b, v2_w, v2_b,
    d1_w, d1_b, d2_w, d2_b, ln1_g, ln1_b, ln2_g, ln2_b,
):
    f = lambda a: np.ascontiguousarray(np.asarray(a), dtype=np.float32)
    x1, x2 = f(input_tensor1), f(input_tensor2)
    m1 = f(attention_mask1).reshape(B, S, 1)
    m2 = f(attention_mask2).reshape(B, S, 1)
    row = lambda a: f(a).reshape(1, HID)

    in_maps = []
    for b in range(B):
        # stream1: ctx1 = attend(q2, k1, v1, mask1); out h1[b]
        in_maps.append({
            "xq": x2[b], "xkv": x1[b],
            "wq": f(q2_w), "wk": f(k1_w), "wv": f(v1_w), "wd": f(d1_w),
            "bq": row(q2_b), "bk": row(k1_b), "bv": row(v1_b), "bd": row(d1_b),
            "mask": m1[b], "lng": row(ln1_g), "lnb": row(ln1_b),
        })
    for b in range(B):
        # stream2: ctx2 = attend(q1, k2, v2, mask2); out h2[b]
        in_maps.append({
            "xq": x1[b], "xkv": x2[b],
            "wq": f(q1_w), "wk": f(k2_w), "wv": f(v2_w), "wd": f(d2_w),
            "bq": row(q1_b), "bk": row(k2_b), "bv": row(v2_b), "bd": row(d2_b),
            "mask": m2[b], "lng": row(ln2_g), "lnb": row(ln2_b),
        })

    return in_maps


def _run(in_maps, **kwargs):
    nc = _get_nc()
    res = bass_utils.run_bass_kernel_spmd(
        nc, in_maps, core_ids=list(range(8)), **kwargs
    )
    h1 = np.stack([res.results[b]["out"] for b in range(B)])
    h2 = np.stack([res.results[B + b]["out"] for b in range(B)])
    return (h1, h2), res


def kernel(**inputs):
    (h1, h2), _ = _run(_prepare(**inputs))
    return h1, h2
